# revision 22
# baseline (speedup 1.0000x reference)
"""Trainium2 Bass kernel for nn_MultiHeadAttention_7413113553038.

Sharding: 8 cores = (batch b in {0,1}) x (query block of 512). Each core
computes all 4 heads of attention for its 512 queries against the full 2048
keys of its batch, plus the output projection, residual add and LayerNorm for
its rows. No collectives needed.

Per-core strategy:
  - Q/K/V activations and the QKV projection weights ship as fp8e4m3 (values
    are O(1); the quantization noise averages out over the 2048-key softmax
    sum), halving the startup-critical DMA bytes. The 1/sqrt(d_k) score
    scale is applied via the EXP activation's scale parameter instead of
    being folded into W_Q (whose values would underflow fp8). The residual,
    fc weights, gauss tables and output stay f16; attention compute is f16.
  - DMAs are issued across the two HWDGE rings (sync/scalar) in need-order,
    smallest-first, split per K/V block so each block's completion semaphore
    fires as early as possible (limits data-starvation when HBM is busy).
  - Q^T/K^T computed in [d, seq] layout (lhsT = W, rhs = X^T); scores
    computed transposed: sT[k, q] = K Q^T per head (contraction d=64, head
    pairs at partition bases 0/64).
  - p = exp(sT) on the scalar engine (f32 psum in -> f16 out, no max
    subtraction; scores are O(6)), multiplicative Gaussian band tables
    E = exp(bias) applied on the 6 rolled k-chunk slots covering the causal
    band (X_K/X_V rolled by q0-256 so the band sits on static slots 0..5).
  - V is augmented with a ones-column so ctxT = V_aug.T @ p accumulates the
    softmax denominator Z as psum row 64 for free.
  - Z handling: Z rows copied to partition 0 (scalar engine), one fused
    custom-DVE reciprocal straight to f16, then a single ones-column matmul
    per head broadcasts 1/Z to the head's 64 partitions (tile_position picks
    the partition base) and one DVE multiply scales ctxT in place.
  - fc: ctxT [dm, q] is the lhsT the fc matmul needs; the residual (+ group-0
    fc partials) is accumulated into the fc psum with an identity-matmul, and
    the LayerNorm normalize runs on the scalar engine as an Identity
    activation with per-row scale=rstd, bias=-mean*rstd (the sqrt table load
    slots naturally into the ACT-idle window after the last EXP).
  - Both attention loops are software-pipelined by one chunk (scores for
    kc+1 are emitted before PV of kc) so interleaved projection matmuls
    never sit between a scores matmul and its EXP in the PE queue; group 1
    carries the K projections it needs plus group 0's dripped epilogue in
    its PE slack. Group 1's first five score/exp chunks are emitted ahead
    of group 0's psum drain so the drain hides behind the EXP stream.
    Output is f16 (upcast on host), two DMAs issued per LayerNorm half.
"""

import numpy as np

N_HEADS = 4
D_K = 64
B = 2
S = 2048
F = 256
QB = 512  # queries per core
P = 128
KC = S // P  # 16 k-chunks
SIGMA_HS = (5.0, 10.0, 20.0, 40.0)
LN_EPS = 1e-5
N_CORES = 8
# per-head causal-bias band width (g >= ~1e-4): ceil(4.292 * sigma)
BAND = (22, 43, 86, 172)
E01_W = 192
E25_W = 304


_CACHE = {}


def _gauss_tables():
    """Multiplicative Gaussian-bias band tables E = exp(g) in fp16,
    transposed-score layout (delta = q - k = off_t + j - i, off_t = 256-128t).

      e01 [4,128,192]: e01[h,i,m] = exp(g_h(m - i + 128)), slots 0,1
                       (slice col = (128 - 128t) + j)
      e25 [4,128,304]: e25[h,i,m] = exp(g_h(m - i)), slots 2..5
                       (slice col = j - 128*(t-2))
    g_h(d) = exp(-d^2 / (2 sigma_h^2)) for d >= 0 else 0.
    """
    i = np.arange(P, dtype=np.float64)[None, :, None]
    sig = np.asarray(SIGMA_HS, dtype=np.float64)[:, None, None]

    m01 = np.arange(E01_W, dtype=np.float64)[None, None, :]
    d01 = m01 - i + 128.0
    g01 = np.where(d01 >= 0, np.exp(-(d01 ** 2) / (2 * sig ** 2)), 0.0)

    m25 = np.arange(E25_W, dtype=np.float64)[None, None, :]
    d25 = m25 - i
    g25 = np.where(d25 >= 0, np.exp(-(d25 ** 2) / (2 * sig ** 2)), 0.0)
    return (
        np.exp(g01).astype(np.float16),
        np.exp(g25).astype(np.float16),
    )


def _build_program():
    import concourse.bass as bass  # noqa: F401
    import concourse.tile as tile
    from concourse import bacc, mybir
    from concourse.dve_ops import (
        RECIP_APPROX_FAST_CONSTS,
        RECIPROCAL_APPROX_FAST,
    )
    from concourse.masks import make_identity

    f32 = mybir.dt.float32
    f16 = mybir.dt.float16
    f8 = mybir.dt.float8e4
    AF = mybir.ActivationFunctionType
    ALU = mybir.AluOpType

    nc = bacc.Bacc("TRN2", target_bir_lowering=False, debug=False)

    # pre-packed inputs; K/V side in fp8
    xkt = nc.dram_tensor("xkt", [P, 4, 2, 512], f8, kind="ExternalInput").ap()
    xqt = nc.dram_tensor("xqt", [P, 2, QB], f8, kind="ExternalInput").ap()
    g01 = nc.dram_tensor("g01", [P, N_HEADS, E01_W], f16, kind="ExternalInput").ap()
    g25 = nc.dram_tensor("g25", [P, N_HEADS, E25_W], f16, kind="ExternalInput").ap()
    res = nc.dram_tensor("res", [P, 4, F], f16, kind="ExternalInput").ap()
    w8 = nc.dram_tensor("w8", [P, 3, 2, F], f8, kind="ExternalInput").ap()
    wf = nc.dram_tensor("wf", [P, 2, F], f16, kind="ExternalInput").ap()
    xvt = nc.dram_tensor("xvt", [P, 4, 2, 512], f8, kind="ExternalInput").ap()
    out = nc.dram_tensor("out", [P, 4, F], f16, kind="ExternalOutput").ap()

    with tile.TileContext(nc) as tc:
        with (
            tc.tile_pool(name="xin", bufs=1) as xin,
            tc.tile_pool(name="proj", bufs=1) as proj,
            tc.tile_pool(name="mmps", bufs=2, space="PSUM") as mmps,
            tc.tile_pool(name="spsum", bufs=2, space="PSUM") as spsum,
            tc.tile_pool(name="cpsum", bufs=2, space="PSUM") as cpsum,
            tc.tile_pool(name="ptpool", bufs=6) as ptpool,
        ):
            # ---- input loads: two HWDGE rings, need-order ----
            xqt_sb = xin.tile([P, 2, QB], f8, tag="xqt")
            nc.sync.dma_start(xqt_sb, xqt)
            xk0_sb = xin.tile([P, 2, 512], f8, tag="xk0")
            nc.sync.dma_start(xk0_sb, xkt[:, 0])
            g01_sb = xin.tile([P, N_HEADS, E01_W], f16, tag="g01")
            nc.sync.dma_start(g01_sb, g01)
            xk123_sb = xin.tile([P, 3, 2, 512], f8, tag="xk123")
            nc.sync.dma_start(xk123_sb[:, 0], xkt[:, 1])
            g25_sb = xin.tile([P, N_HEADS, E25_W], f16, tag="g25")
            nc.sync.dma_start(g25_sb, g25)
            nc.sync.dma_start(xk123_sb[:, 1], xkt[:, 2])
            nc.sync.dma_start(xk123_sb[:, 2], xkt[:, 3])
            res_sb = xin.tile([P, 4, F], f16, tag="res")
            nc.sync.dma_start(res_sb, res)

            w8_sb = xin.tile([P, 3, 2, F], f8, tag="w8")
            nc.scalar.dma_start(w8_sb[:, 0:2], w8[:, 0:2])
            nc.scalar.dma_start(w8_sb[:, 2:3], w8[:, 2:3])
            xv0_sb = xin.tile([P, 2, 512], f8, tag="xv0")
            nc.scalar.dma_start(xv0_sb, xvt[:, 0])
            wf_sb = xin.tile([P, 2, F], f16, tag="wf")
            nc.scalar.dma_start(wf_sb, wf)
            xv123_sb = xin.tile([P, 3, 2, 512], f8, tag="xv123")
            nc.scalar.dma_start(xv123_sb[:, 0], xvt[:, 1])
            nc.scalar.dma_start(xv123_sb[:, 1], xvt[:, 2])
            nc.scalar.dma_start(xv123_sb[:, 2], xvt[:, 3])

            # ---- persistent tiles ----
            qt_sb = proj.tile([P, 2, QB], f16, tag="qt")
            kt_sb = proj.tile([P, 4, 2, 512], f16, tag="kt")
            v_sb = proj.tile([P, KC, N_HEADS, 65], f16, tag="v")
            ctx_sb = proj.tile([P, 2, QB], f16, tag="ctx")
            fcacc = proj.tile([P, 4, F], f16, tag="fcacc")
            o_sb = proj.tile([P, 4, F], f16, tag="osb")
            ztmp32 = proj.tile([1, N_HEADS, QB], f32, tag="ztmp32")
            rz16 = proj.tile([1, N_HEADS, QB], f16, tag="rz16")
            ones16 = proj.tile([1, 64], f16, tag="ones16")
            ident = proj.tile([P, P], f16, tag="ident")
            eps_t = proj.tile([P, 1], f32, tag="eps")
            st_t = proj.tile([P, 4, 6], f32, tag="st")
            mv_t = proj.tile([P, 4, 2], f32, tag="mv")
            rstd = proj.tile([P, 4], f32, tag="rstd")
            nbias = proj.tile([P, 4], f32, tag="nbias")
            negone = proj.tile([P, 1], f32, tag="negone")

            nc.vector.memset(ones16, 1.0)
            nc.vector.memset(eps_t, LN_EPS)
            nc.vector.memset(negone, -1.0)
            nc.vector.memset(v_sb[:, :, :, 64:65], 1.0)
            make_identity(nc, ident)  # gpsimd-only


            # ---- projection helpers ----
            def proj_k(nb, g, split_cast=False):
                ps = mmps.tile([P, 512], f32, tag="mm", name=f"psk{nb}{g}")
                xk = xk0_sb if nb == 0 else xk123_sb[:, nb - 1]
                for c in range(2):
                    nc.tensor.matmul(
                        ps,
                        w8_sb[:, 0, c, g * P:(g + 1) * P],
                        xk[:, c, :],
                        start=(c == 0),
                        stop=(c == 1),
                    )
                if split_cast:
                    nc.vector.tensor_copy(kt_sb[:, nb, g, 0:P], ps[:, 0:P])
                    nc.vector.tensor_copy(kt_sb[:, nb, g, P:], ps[:, P:])
                else:
                    nc.vector.tensor_copy(kt_sb[:, nb, g, :], ps)

            def proj_q(g, on_scalar=False):
                ps = mmps.tile([P, 512], f32, tag="mm", name=f"psq{g}")
                for c in range(2):
                    nc.tensor.matmul(
                        ps,
                        w8_sb[:, 1, c, g * P:(g + 1) * P],
                        xqt_sb[:, c, :],
                        start=(c == 0),
                        stop=(c == 1),
                    )
                if on_scalar:
                    nc.scalar.copy(qt_sb[:, g, :], ps)
                else:
                    nc.vector.tensor_copy(qt_sb[:, g, :], ps)

            def proj_v(j):
                nb, jj = divmod(j, 4)
                ps = mmps.tile([P, 512], f32, tag="mm", name=f"psv{j}")
                psv = ps[:, :F]
                xv = xv0_sb if nb == 0 else xv123_sb[:, nb - 1]
                for c in range(2):
                    nc.tensor.matmul(
                        psv,
                        xv[:, c, jj * P:(jj + 1) * P],
                        w8_sb[:, 2, c, :],
                        start=(c == 0),
                        stop=(c == 1),
                    )
                nc.vector.tensor_copy(
                    v_sb[:, j, :, 0:64],
                    psv.rearrange("p (h d) -> p h d", h=N_HEADS),
                )

            # ---- attention ----
            def attn_sc(G, kc):
                ps = spsum.tile([P, 2 * QB], f32, tag="sc", name=f"sc{G[0]}_{kc}")
                for hi, h in enumerate(G):
                    g, po = h // 2, (h % 2) * 64
                    nc.tensor.matmul(
                        ps[:, hi * QB:(hi + 1) * QB],
                        kt_sb[po:po + 64, kc // 4, g, (kc % 4) * P:(kc % 4 + 1) * P],
                        qt_sb[po:po + 64, g, :],
                        start=True,
                        stop=True,
                    )
                pt = ptpool.tile([P, 2, QB], f16, tag="pt", name=f"pt{G[0]}_{kc}")
                nc.scalar.activation(
                    pt.rearrange("p a b -> p (a b)"), ps, AF.Exp,
                    scale=0.125,
                )
                return pt

            def attn_band(G, kc, pt):
                if kc > 5:
                    return
                for hi, h in enumerate(G):
                    off_t = 256 - 128 * kc
                    j0 = max(0, -off_t)
                    j1 = min(512, BAND[h] + 128 - off_t)
                    j1 = min(512, (j1 + 7) & ~7)
                    if j1 > j0:
                        if kc <= 1:
                            c0 = (128 - 128 * kc) + j0
                            esl = g01_sb[:, h, c0:c0 + (j1 - j0)]
                        else:
                            c0 = j0 - 128 * (kc - 2)
                            esl = g25_sb[:, h, c0:c0 + (j1 - j0)]
                        nc.vector.tensor_mul(
                            pt[:, hi, j0:j1], pt[:, hi, j0:j1], esl
                        )

            def attn_pv(G, ctxps, kc, pt):
                for hi, h in enumerate(G):
                    nc.tensor.matmul(
                        ctxps[hi][0:65, :],
                        v_sb[:, kc, h, :],
                        pt[:, hi, :],
                        start=(kc == 0),
                        stop=(kc == KC - 1),
                    )

            def attn_kc(G, ctxps, kc):
                pt = attn_sc(G, kc)
                attn_band(G, kc, pt)
                attn_pv(G, ctxps, kc, pt)

            # ---- epilogue pieces ----
            def e_zrows(G, ctxps, on_scalar=True):
                """Z rows (psum partition 64) -> partition-0 f32 SBUF."""
                for hi, h in enumerate(G):
                    if on_scalar:
                        nc.scalar.copy(ztmp32[0:1, h, :], ctxps[hi][64:65, :])
                    else:
                        nc.vector.tensor_copy(
                            ztmp32[0:1, h, :], ctxps[hi][64:65, :]
                        )

            def e_ctxcopy(G, ctxps):
                gg = G[0] // 2
                for hi, h in enumerate(G):
                    po = (h % 2) * 64
                    nc.vector.tensor_copy(
                        ctx_sb[po:po + 64, gg, :], ctxps[hi][0:64, :]
                    )

            def e_recip(G):
                c = RECIP_APPROX_FAST_CONSTS
                for hi, h in enumerate(G):
                    nc.vector._custom_dve(
                        RECIPROCAL_APPROX_FAST,
                        out=rz16[0:1, h, :],
                        in0=ztmp32[0:1, h, :],
                        s0=c["s0"],
                        s1=c["s1"],
                        imm2=c["imm2"],
                    )

            def e_zscale(G, pool=None):
                """Broadcast 1/Z to each head's partitions and scale ctx."""
                gg = G[0] // 2
                if pool is None:
                    zb = mmps.tile([P, 512], f32, tag="mm", name=f"zb{gg}")
                else:
                    zb = pool.tile([P, QB], f32, tag="ctxp", name=f"zb{gg}")
                for hi, h in enumerate(G):
                    po = (h % 2) * 64
                    nc.tensor.matmul(
                        zb[po:po + 64, :],
                        ones16[0:1, :],
                        rz16[0:1, h, :],
                        start=True,
                        stop=True,
                        tile_position=(0, po),
                    )
                for hi, h in enumerate(G):
                    po = (h % 2) * 64
                    nc.vector.tensor_mul(
                        ctx_sb[po:po + 64, gg, :],
                        ctx_sb[po:po + 64, gg, :],
                        zb[po:po + 64, :],
                    )

            def e_fc0_pair(p_):
                """G0 fc for qc pair p_: psum -> fcacc (f16 SBUF)."""
                ps = mmps.tile([P, 512], f32, tag="mm", name=f"fc0p{p_}")
                for i in range(2):
                    qc = 2 * p_ + i
                    nc.tensor.matmul(
                        ps[:, i * F:(i + 1) * F],
                        ctx_sb[:, 0, qc * P:(qc + 1) * P],
                        wf_sb[:, 0, :],
                        start=True,
                        stop=True,
                    )
                nc.vector.tensor_add(
                    fcacc[:, 2 * p_:2 * p_ + 2, :].rearrange("p a b -> p (a b)"),
                    fcacc[:, 2 * p_:2 * p_ + 2, :].rearrange("p a b -> p (a b)"),
                    ps,
                )

            def e_fc1_mm(p_):
                """G1 fc + residual/fcacc via identity matmul, one qc pair.
                The ident (residual) matmul leads the group: its data is
                ready before the 1/Z scales, so PE isn't gated on them."""
                ps = mmps.tile([P, 512], f32, tag="mm", name=f"fc1p{p_}")
                for i in range(2):
                    qc = 2 * p_ + i
                    sl = ps[:, i * F:(i + 1) * F]
                    nc.tensor.matmul(
                        sl,
                        ident,
                        fcacc[:, qc, :],
                        start=True,
                        stop=False,
                        skip_group_check=True,
                    )
                    nc.tensor.matmul(
                        sl,
                        ctx_sb[:, 1, qc * P:(qc + 1) * P],
                        wf_sb[:, 1, :],
                        start=False,
                        stop=True,
                        skip_group_check=True,
                    )
                return ps

            def e_fc1_ln(p_, ps, on_scalar):
                """LayerNorm one qc pair off the fc psum; normalize on ACT
                (Identity w/ scale+bias) or DVE (tensor_scalar)."""
                for i in range(2):
                    qc = 2 * p_ + i
                    sl = ps[:, i * F:(i + 1) * F]
                    nc.vector.bn_stats(st_t[:, qc, :], sl)
                    nc.vector.bn_aggr(mv_t[:, qc, :], st_t[:, qc, :])
                q0 = 2 * p_
                nc.scalar.activation(
                    rstd[:, q0:q0 + 2], mv_t[:, q0:q0 + 2, 1],
                    AF.Sqrt, bias=eps_t, scale=1.0,
                )
                nc.vector.reciprocal(rstd[:, q0:q0 + 2], rstd[:, q0:q0 + 2])
                if on_scalar:
                    nc.vector.tensor_mul(
                        nbias[:, q0:q0 + 2], mv_t[:, q0:q0 + 2, 0],
                        rstd[:, q0:q0 + 2],
                    )
                    nc.vector.tensor_scalar_mul(
                        nbias[:, q0:q0 + 2], nbias[:, q0:q0 + 2], negone
                    )
                for i in range(2):
                    qc = 2 * p_ + i
                    if on_scalar:
                        nc.scalar.activation(
                            o_sb[:, qc, :],
                            ps[:, i * F:(i + 1) * F],
                            AF.Identity,
                            bias=nbias[:, qc:qc + 1],
                            scale=rstd[:, qc:qc + 1],
                        )
                    else:
                        nc.vector.tensor_scalar(
                            o_sb[:, qc, :],
                            ps[:, i * F:(i + 1) * F],
                            mv_t[:, qc, 0:1],
                            rstd[:, qc:qc + 1],
                            op0=ALU.subtract,
                            op1=ALU.mult,
                        )
                eng = nc.sync if p_ == 0 else nc.scalar
                eng.dma_start(
                    out[:, q0:q0 + 2, :], o_sb[:, q0:q0 + 2, :]
                )

            # ---- G0: prologue + software-pipelined loop (scores for kc+1
            # are emitted before PV of kc so dripped projections never sit
            # between a scores matmul and its EXP on the PE queue) ----
            G0, G1 = (0, 1), (2, 3)
            ctxps0 = [
                cpsum.tile([P, QB], f32, tag="ctxp", name=f"ctxp{hh}")
                for hh in G0
            ]
            proj_q(0, on_scalar=True)
            proj_k(0, 0, split_cast=True)
            pt_cur = attn_sc(G0, 0)
            attn_band(G0, 0, pt_cur)
            proj_v(0)
            proj_v(1)
            proj_v(2)

            post = {
                0: [lambda: proj_v(3)],
                1: [lambda: proj_k(1, 0)],
                2: [lambda: proj_v(4), lambda: proj_v(5)],
                3: [lambda: proj_v(6)],
                4: [lambda: proj_v(7), lambda: proj_k(2, 0)],
                5: [lambda: proj_v(8), lambda: proj_v(9)],
                6: [lambda: proj_q(1), lambda: proj_v(10)],
                7: [lambda: proj_v(11), lambda: proj_k(3, 0)],
                8: [lambda: proj_v(12)],
                9: [lambda: proj_v(13), lambda: proj_k(0, 1)],
                10: [lambda: proj_v(14)],
                11: [lambda: proj_v(15), lambda: proj_k(1, 1)],
            }
            for kc in range(KC):
                if kc + 1 < KC:
                    pt_nxt = attn_sc(G0, kc + 1)
                    attn_band(G0, kc + 1, pt_nxt)
                attn_pv(G0, ctxps0, kc, pt_cur)
                pt_cur = pt_nxt
                for step in post.get(kc, []):
                    step()

            # ---- G0 -> G1 transition: 5-chunk score/exp window over the
            # drain so the psum handoff hides behind the EXP stream ----
            pts = []
            for kc in range(5):
                pt = attn_sc(G1, kc)
                attn_band(G1, kc, pt)
                pts.append(pt)
                if kc == 1:
                    e_zrows(G0, ctxps0, on_scalar=False)
                elif kc == 2:
                    e_ctxcopy(G0, ctxps0)
                elif kc == 3:
                    e_recip(G0)
            ctxps1 = [
                cpsum.tile([P, QB], f32, tag="ctxp", name=f"ctxp{hh}")
                for hh in G1
            ]
            for kc in range(4):
                attn_pv(G1, ctxps1, kc, pts[kc])
            pt_cur = pts[4]

            # ---- G1 pipelined loop with G0-epilogue drip ----
            drip = {
                6: [lambda: proj_k(2, 1)],
                7: [lambda: e_zscale(G0)],
                8: [lambda: proj_k(3, 1)],
                9: [lambda: nc.vector.tensor_copy(fcacc, res_sb)],
                11: [lambda: e_fc0_pair(0)],
                13: [lambda: e_fc0_pair(1)],
            }
            for kc in range(4, KC):
                if kc + 1 < KC:
                    pt_nxt = attn_sc(G1, kc + 1)
                    attn_band(G1, kc + 1, pt_nxt)
                attn_pv(G1, ctxps1, kc, pt_cur)
                pt_cur = pt_nxt
                for step in drip.get(kc, []):
                    step()

            # ---- G1 epilogue ----
            e_zrows(G1, ctxps1)
            c_ = RECIP_APPROX_FAST_CONSTS
            nc.vector._custom_dve(
                RECIPROCAL_APPROX_FAST, out=rz16[0:1, 2, :],
                in0=ztmp32[0:1, 2, :],
                s0=c_["s0"], s1=c_["s1"], imm2=c_["imm2"],
            )
            nc.vector.tensor_copy(ctx_sb[0:64, 1, :], ctxps1[0][0:64, :])
            nc.scalar.copy(ctx_sb[64:128, 1, :], ctxps1[1][0:64, :])
            nc.vector._custom_dve(
                RECIPROCAL_APPROX_FAST, out=rz16[0:1, 3, :],
                in0=ztmp32[0:1, 3, :],
                s0=c_["s0"], s1=c_["s1"], imm2=c_["imm2"],
            )
            e_zscale(G1, pool=cpsum)
            ps0 = e_fc1_mm(0)
            ps1 = e_fc1_mm(1)
            e_fc1_ln(0, ps0, on_scalar=True)
            e_fc1_ln(1, ps1, on_scalar=False)

    nc.compile()
    return nc


def get_nc():
    if "nc" not in _CACHE:
        _CACHE["nc"] = _build_program()
    return _CACHE["nc"]


def make_in_maps(input_Q, input_K, input_V, W_Q, W_K, W_V, W_fc):
    import ml_dtypes

    f8 = ml_dtypes.float8_e4m3
    c16 = lambda a: np.ascontiguousarray(
        np.asarray(a, dtype=np.float32), dtype=np.float16
    )
    # pack an [in, out] matrix to SBUF layout [p, c, out]
    pk_w = lambda w: np.asarray(w, np.float32).reshape(2, P, -1).transpose(1, 0, 2)
    # pack an activation block X [seq, F] to X^T SBUF layout [p, c, seq]
    pk_t = lambda x: c16(np.asarray(x, np.float32).T.reshape(2, P, -1).transpose(1, 0, 2))
    # pack a rolled key/value matrix [2048, F] to X^T [p, nb, c, 512]
    pk_x = lambda x: np.ascontiguousarray(
        np.asarray(x, np.float32).reshape(4, 512, 2, P).transpose(3, 0, 2, 1),
        dtype=f8,
    )
    e01t, e25t = _gauss_tables()
    g01 = np.ascontiguousarray(e01t.transpose(1, 0, 2))
    g25 = np.ascontiguousarray(e25t.transpose(1, 0, 2))
    g01_neutral = np.ones_like(g01)
    w8 = np.ascontiguousarray(
        np.stack([pk_w(W_K), pk_w(W_Q), pk_w(W_V)], axis=1), dtype=f8
    )
    wf = c16(pk_w(W_fc))
    in_maps = []
    for c in range(N_CORES):
        b, qb = divmod(c, 4)
        q0 = qb * QB
        r = (q0 - 256) % S
        xq_blk = np.asarray(input_Q[b][q0:q0 + QB], np.float32)
        xk_rot = np.roll(np.asarray(input_K[b], np.float32), -r, axis=0)
        xv_rot = np.roll(np.asarray(input_V[b], np.float32), -r, axis=0)
        in_maps.append({
            "xkt": pk_x(xk_rot),
            "xqt": np.ascontiguousarray(pk_t(xq_blk), dtype=f8),
            "g01": g01_neutral if q0 == 0 else g01,
            "g25": g25,
            "res": c16(xq_blk.reshape(4, P, F).transpose(1, 0, 2)),
            "w8": w8,
            "wf": wf,
            "xvt": pk_x(xv_rot),
        })
    return in_maps


def assemble_out(results):
    out = np.empty((B, S, F), dtype=np.float32)
    for c in range(N_CORES):
        b, qb = divmod(c, 4)
        o = np.asarray(results[c]["out"], dtype=np.float32)
        out[b, qb * QB:(qb + 1) * QB, :] = o.transpose(1, 0, 2).reshape(QB, F)
    return out


def kernel(input_Q, input_K, input_V, W_Q, W_K, W_V, W_fc, attn_mask=None):
    from concourse.bass_utils import run_bass_kernel_spmd

    nc = get_nc()
    in_maps = make_in_maps(input_Q, input_K, input_V, W_Q, W_K, W_V, W_fc)
    res = run_bass_kernel_spmd(nc, in_maps, core_ids=list(range(N_CORES)))
    return assemble_out(res.results)


# revision 23
# speedup vs baseline: 1.0168x; 1.0168x over previous
"""Trainium2 Bass kernel for nn_MultiHeadAttention_7413113553038.

Sharding: 8 cores = (batch b in {0,1}) x (query block of 512). Each core
computes all 4 heads of attention for its 512 queries against the full 2048
keys of its batch, plus the output projection, residual add and LayerNorm for
its rows. No collectives needed.

Per-core strategy:
  - Q/K/V activations and the QKV projection weights ship as fp8e4m3 (values
    are O(1); the quantization noise averages out over the 2048-key softmax
    sum), halving the startup-critical DMA bytes. The 1/sqrt(d_k) score
    scale is applied via the EXP activation's scale parameter instead of
    being folded into W_Q (whose values would underflow fp8). The residual,
    fc weights, gauss tables and output stay f16; attention compute is f16.
  - DMAs are issued across the two HWDGE rings (sync/scalar) in need-order,
    smallest-first, split per K/V block so each block's completion semaphore
    fires as early as possible (limits data-starvation when HBM is busy).
  - Q^T/K^T computed in [d, seq] layout (lhsT = W, rhs = X^T); scores
    computed transposed: sT[k, q] = K Q^T per head (contraction d=64, head
    pairs at partition bases 0/64).
  - p = exp(sT) on the scalar engine (f32 psum in -> f16 out, no max
    subtraction; scores are O(6)), multiplicative Gaussian band tables
    E = exp(bias) applied on the 6 rolled k-chunk slots covering the causal
    band (X_K/X_V rolled by q0-256 so the band sits on static slots 0..5).
  - V is augmented with a ones-column so ctxT = V_aug.T @ p accumulates the
    softmax denominator Z as psum row 64 for free.
  - Z handling: Z rows copied to partition 0 (scalar engine), one fused
    custom-DVE reciprocal straight to f16, then a single ones-column matmul
    per head broadcasts 1/Z to the head's 64 partitions (tile_position picks
    the partition base) and one DVE multiply scales ctxT in place.
  - fc: ctxT [dm, q] is the lhsT the fc matmul needs; the residual (+ group-0
    fc partials) is accumulated into the fc psum with an identity-matmul, and
    the LayerNorm normalize runs on the scalar engine as an Identity
    activation with per-row scale=rstd, bias=-mean*rstd (the sqrt table load
    slots naturally into the ACT-idle window after the last EXP).
  - Both attention loops are software-pipelined by one chunk (scores for
    kc+1 are emitted before PV of kc) so interleaved projection matmuls
    never sit between a scores matmul and its EXP in the PE queue; group 1
    carries the K projections it needs plus group 0's dripped epilogue in
    its PE slack. Group 1's first five score/exp chunks are emitted ahead
    of group 0's psum drain so the drain hides behind the EXP stream.
    Output is f16 (upcast on host), two DMAs issued per LayerNorm half.
"""

import numpy as np

N_HEADS = 4
D_K = 64
B = 2
S = 2048
F = 256
QB = 512  # queries per core
P = 128
KC = S // P  # 16 k-chunks
SIGMA_HS = (5.0, 10.0, 20.0, 40.0)
LN_EPS = 1e-5
N_CORES = 8
# per-head causal-bias band width (g >= ~1e-4): ceil(4.292 * sigma)
BAND = (22, 43, 86, 172)
E01_W = 192
E25_W = 304


_CACHE = {}


def _gauss_tables():
    """Multiplicative Gaussian-bias band tables E = exp(g) in fp16,
    transposed-score layout (delta = q - k = off_t + j - i, off_t = 256-128t).

      e01 [4,128,192]: e01[h,i,m] = exp(g_h(m - i + 128)), slots 0,1
                       (slice col = (128 - 128t) + j)
      e25 [4,128,304]: e25[h,i,m] = exp(g_h(m - i)), slots 2..5
                       (slice col = j - 128*(t-2))
    g_h(d) = exp(-d^2 / (2 sigma_h^2)) for d >= 0 else 0.
    """
    i = np.arange(P, dtype=np.float64)[None, :, None]
    sig = np.asarray(SIGMA_HS, dtype=np.float64)[:, None, None]

    m01 = np.arange(E01_W, dtype=np.float64)[None, None, :]
    d01 = m01 - i + 128.0
    g01 = np.where(d01 >= 0, np.exp(-(d01 ** 2) / (2 * sig ** 2)), 0.0)

    m25 = np.arange(E25_W, dtype=np.float64)[None, None, :]
    d25 = m25 - i
    g25 = np.where(d25 >= 0, np.exp(-(d25 ** 2) / (2 * sig ** 2)), 0.0)
    return (
        np.exp(g01).astype(np.float16),
        np.exp(g25).astype(np.float16),
    )


def _build_program():
    import concourse.bass as bass  # noqa: F401
    import concourse.tile as tile
    from concourse import bacc, mybir
    from concourse.dve_ops import (
        RECIP_APPROX_FAST_CONSTS,
        RECIPROCAL_APPROX_FAST,
    )
    from concourse.masks import make_identity

    f32 = mybir.dt.float32
    f16 = mybir.dt.float16
    f8 = mybir.dt.float8e4
    AF = mybir.ActivationFunctionType
    ALU = mybir.AluOpType

    nc = bacc.Bacc("TRN2", target_bir_lowering=False, debug=False)

    # pre-packed inputs; K/V side in fp8
    xkt = nc.dram_tensor("xkt", [P, 4, 2, 512], f8, kind="ExternalInput").ap()
    xqt = nc.dram_tensor("xqt", [P, 2, QB], f8, kind="ExternalInput").ap()
    g01 = nc.dram_tensor("g01", [P, N_HEADS, E01_W], f16, kind="ExternalInput").ap()
    g25 = nc.dram_tensor("g25", [P, N_HEADS, E25_W], f16, kind="ExternalInput").ap()
    res = nc.dram_tensor("res", [P, 4, F], f16, kind="ExternalInput").ap()
    w8 = nc.dram_tensor("w8", [P, 3, 2, F], f8, kind="ExternalInput").ap()
    wf = nc.dram_tensor("wf", [P, 2, F], f16, kind="ExternalInput").ap()
    xvt = nc.dram_tensor("xvt", [P, 4, 2, 512], f8, kind="ExternalInput").ap()
    out = nc.dram_tensor("out", [P, 4, F], f16, kind="ExternalOutput").ap()

    with tile.TileContext(nc) as tc:
        with (
            tc.tile_pool(name="xin", bufs=1) as xin,
            tc.tile_pool(name="proj", bufs=1) as proj,
            tc.tile_pool(name="mmps", bufs=2, space="PSUM") as mmps,
            tc.tile_pool(name="spsum", bufs=2, space="PSUM") as spsum,
            tc.tile_pool(name="cpsum", bufs=2, space="PSUM") as cpsum,
            tc.tile_pool(name="ptpool", bufs=6) as ptpool,
        ):
            # ---- input loads: two HWDGE rings, need-order ----
            xqt_sb = xin.tile([P, 2, QB], f8, tag="xqt")
            nc.sync.dma_start(xqt_sb, xqt)
            xk0_sb = xin.tile([P, 2, 512], f8, tag="xk0")
            nc.sync.dma_start(xk0_sb, xkt[:, 0])
            g01_sb = xin.tile([P, N_HEADS, E01_W], f16, tag="g01")
            nc.sync.dma_start(g01_sb, g01)
            xk123_sb = xin.tile([P, 3, 2, 512], f8, tag="xk123")
            nc.sync.dma_start(xk123_sb[:, 0], xkt[:, 1])
            g25_sb = xin.tile([P, N_HEADS, E25_W], f16, tag="g25")
            nc.sync.dma_start(g25_sb, g25)
            nc.sync.dma_start(xk123_sb[:, 1], xkt[:, 2])
            nc.sync.dma_start(xk123_sb[:, 2], xkt[:, 3])
            res_sb = xin.tile([P, 4, F], f16, tag="res")
            nc.sync.dma_start(res_sb, res)

            w8_sb = xin.tile([P, 3, 2, F], f8, tag="w8")
            nc.scalar.dma_start(w8_sb[:, 0:2], w8[:, 0:2])
            nc.scalar.dma_start(w8_sb[:, 2:3], w8[:, 2:3])
            xv0_sb = xin.tile([P, 2, 512], f8, tag="xv0")
            nc.scalar.dma_start(xv0_sb, xvt[:, 0])
            wf_sb = xin.tile([P, 2, F], f16, tag="wf")
            nc.scalar.dma_start(wf_sb, wf)
            xv123_sb = xin.tile([P, 3, 2, 512], f8, tag="xv123")
            nc.scalar.dma_start(xv123_sb[:, 0], xvt[:, 1])
            nc.scalar.dma_start(xv123_sb[:, 1], xvt[:, 2])
            nc.scalar.dma_start(xv123_sb[:, 2], xvt[:, 3])

            # ---- persistent tiles ----
            qt_sb = proj.tile([P, 2, QB], f16, tag="qt")
            kt_sb = proj.tile([P, 4, 2, 512], f16, tag="kt")
            v_sb = proj.tile([P, KC, N_HEADS, 65], f16, tag="v")
            ctx_sb = proj.tile([P, 2, QB], f16, tag="ctx")
            fcacc = proj.tile([P, 4, F], f16, tag="fcacc")
            o_sb = proj.tile([P, 4, F], f16, tag="osb")
            ztmp32 = proj.tile([1, N_HEADS, QB], f32, tag="ztmp32")
            rz16 = proj.tile([1, N_HEADS, QB], f16, tag="rz16")
            ones16 = proj.tile([1, 64], f16, tag="ones16")
            ident = proj.tile([P, P], f16, tag="ident")
            eps_t = proj.tile([P, 1], f32, tag="eps")
            st_t = proj.tile([P, 4, 6], f32, tag="st")
            mv_t = proj.tile([P, 4, 2], f32, tag="mv")
            rstd = proj.tile([P, 4], f32, tag="rstd")
            nbias = proj.tile([P, 4], f32, tag="nbias")
            negone = proj.tile([P, 1], f32, tag="negone")

            nc.vector.memset(ones16, 1.0)
            nc.vector.memset(eps_t, LN_EPS)
            nc.vector.memset(negone, -1.0)
            nc.vector.memset(v_sb[:, :, :, 64:65], 1.0)
            make_identity(nc, ident)  # gpsimd-only


            # ---- projection helpers ----
            def proj_k(nb, g, split_cast=False):
                ps = mmps.tile([P, 512], f32, tag="mm", name=f"psk{nb}{g}")
                xk = xk0_sb if nb == 0 else xk123_sb[:, nb - 1]
                for c in range(2):
                    nc.tensor.matmul(
                        ps,
                        w8_sb[:, 0, c, g * P:(g + 1) * P],
                        xk[:, c, :],
                        start=(c == 0),
                        stop=(c == 1),
                    )
                if split_cast:
                    nc.vector.tensor_copy(kt_sb[:, nb, g, 0:P], ps[:, 0:P])
                    nc.vector.tensor_copy(kt_sb[:, nb, g, P:], ps[:, P:])
                else:
                    nc.vector.tensor_copy(kt_sb[:, nb, g, :], ps)

            def proj_q(g, on_scalar=False):
                ps = mmps.tile([P, 512], f32, tag="mm", name=f"psq{g}")
                for c in range(2):
                    nc.tensor.matmul(
                        ps,
                        w8_sb[:, 1, c, g * P:(g + 1) * P],
                        xqt_sb[:, c, :],
                        start=(c == 0),
                        stop=(c == 1),
                    )
                if on_scalar:
                    nc.scalar.copy(qt_sb[:, g, :], ps)
                else:
                    nc.vector.tensor_copy(qt_sb[:, g, :], ps)

            def proj_v(j):
                nb, jj = divmod(j, 4)
                ps = mmps.tile([P, 512], f32, tag="mm", name=f"psv{j}")
                psv = ps[:, :F]
                xv = xv0_sb if nb == 0 else xv123_sb[:, nb - 1]
                for c in range(2):
                    nc.tensor.matmul(
                        psv,
                        xv[:, c, jj * P:(jj + 1) * P],
                        w8_sb[:, 2, c, :],
                        start=(c == 0),
                        stop=(c == 1),
                    )
                nc.vector.tensor_copy(
                    v_sb[:, j, :, 0:64],
                    psv.rearrange("p (h d) -> p h d", h=N_HEADS),
                )

            # ---- attention ----
            def attn_sc(G, kc):
                ps = spsum.tile([P, 2 * QB], f32, tag="sc", name=f"sc{G[0]}_{kc}")
                for hi, h in enumerate(G):
                    g, po = h // 2, (h % 2) * 64
                    nc.tensor.matmul(
                        ps[:, hi * QB:(hi + 1) * QB],
                        kt_sb[po:po + 64, kc // 4, g, (kc % 4) * P:(kc % 4 + 1) * P],
                        qt_sb[po:po + 64, g, :],
                        start=True,
                        stop=True,
                    )
                pt = ptpool.tile([P, 2, QB], f16, tag="pt", name=f"pt{G[0]}_{kc}")
                nc.scalar.activation(
                    pt.rearrange("p a b -> p (a b)"), ps, AF.Exp,
                    scale=0.125,
                )
                return pt

            def attn_band(G, kc, pt):
                if kc > 5:
                    return
                for hi, h in enumerate(G):
                    off_t = 256 - 128 * kc
                    j0 = max(0, -off_t)
                    j1 = min(512, BAND[h] + 128 - off_t)
                    j1 = min(512, (j1 + 7) & ~7)
                    if j1 > j0:
                        if kc <= 1:
                            c0 = (128 - 128 * kc) + j0
                            esl = g01_sb[:, h, c0:c0 + (j1 - j0)]
                        else:
                            c0 = j0 - 128 * (kc - 2)
                            esl = g25_sb[:, h, c0:c0 + (j1 - j0)]
                        nc.vector.tensor_mul(
                            pt[:, hi, j0:j1], pt[:, hi, j0:j1], esl
                        )

            def attn_pv(G, ctxps, kc, pt):
                for hi, h in enumerate(G):
                    nc.tensor.matmul(
                        ctxps[hi][0:65, :],
                        v_sb[:, kc, h, :],
                        pt[:, hi, :],
                        start=(kc == 0),
                        stop=(kc == KC - 1),
                    )

            def attn_kc(G, ctxps, kc):
                pt = attn_sc(G, kc)
                attn_band(G, kc, pt)
                attn_pv(G, ctxps, kc, pt)

            # ---- epilogue pieces ----
            def e_zrows(G, ctxps, on_scalar=True):
                """Z rows (psum partition 64) -> partition-0 f32 SBUF."""
                for hi, h in enumerate(G):
                    if on_scalar:
                        nc.scalar.copy(ztmp32[0:1, h, :], ctxps[hi][64:65, :])
                    else:
                        nc.vector.tensor_copy(
                            ztmp32[0:1, h, :], ctxps[hi][64:65, :]
                        )

            def e_ctxcopy(G, ctxps):
                gg = G[0] // 2
                for hi, h in enumerate(G):
                    po = (h % 2) * 64
                    nc.vector.tensor_copy(
                        ctx_sb[po:po + 64, gg, :], ctxps[hi][0:64, :]
                    )

            def e_recip(G):
                c = RECIP_APPROX_FAST_CONSTS
                for hi, h in enumerate(G):
                    nc.vector._custom_dve(
                        RECIPROCAL_APPROX_FAST,
                        out=rz16[0:1, h, :],
                        in0=ztmp32[0:1, h, :],
                        s0=c["s0"],
                        s1=c["s1"],
                        imm2=c["imm2"],
                    )

            def e_zscale(G, pool=None):
                """Broadcast 1/Z to each head's partitions and scale ctx."""
                gg = G[0] // 2
                if pool is None:
                    zb = mmps.tile([P, 512], f32, tag="mm", name=f"zb{gg}")
                else:
                    zb = pool.tile([P, QB], f32, tag="ctxp", name=f"zb{gg}")
                for hi, h in enumerate(G):
                    po = (h % 2) * 64
                    nc.tensor.matmul(
                        zb[po:po + 64, :],
                        ones16[0:1, :],
                        rz16[0:1, h, :],
                        start=True,
                        stop=True,
                        tile_position=(0, po),
                    )
                for hi, h in enumerate(G):
                    po = (h % 2) * 64
                    nc.vector.tensor_mul(
                        ctx_sb[po:po + 64, gg, :],
                        ctx_sb[po:po + 64, gg, :],
                        zb[po:po + 64, :],
                    )

            def e_fc0_pair(p_):
                """G0 fc for qc pair p_: psum -> fcacc (f16 SBUF)."""
                ps = mmps.tile([P, 512], f32, tag="mm", name=f"fc0p{p_}")
                for i in range(2):
                    qc = 2 * p_ + i
                    nc.tensor.matmul(
                        ps[:, i * F:(i + 1) * F],
                        ctx_sb[:, 0, qc * P:(qc + 1) * P],
                        wf_sb[:, 0, :],
                        start=True,
                        stop=True,
                    )
                nc.vector.tensor_add(
                    fcacc[:, 2 * p_:2 * p_ + 2, :].rearrange("p a b -> p (a b)"),
                    fcacc[:, 2 * p_:2 * p_ + 2, :].rearrange("p a b -> p (a b)"),
                    ps,
                )

            def e_fc1_mm(p_):
                """G1 fc + residual/fcacc via identity matmul, one qc pair.
                The ident (residual) matmul leads the group: its data is
                ready before the 1/Z scales, so PE isn't gated on them."""
                ps = mmps.tile([P, 512], f32, tag="mm", name=f"fc1p{p_}")
                for i in range(2):
                    qc = 2 * p_ + i
                    sl = ps[:, i * F:(i + 1) * F]
                    nc.tensor.matmul(
                        sl,
                        ident,
                        fcacc[:, qc, :],
                        start=True,
                        stop=False,
                        skip_group_check=True,
                    )
                    nc.tensor.matmul(
                        sl,
                        ctx_sb[:, 1, qc * P:(qc + 1) * P],
                        wf_sb[:, 1, :],
                        start=False,
                        stop=True,
                        skip_group_check=True,
                    )
                return ps

            def e_fc1_ln(p_, ps, on_scalar):
                """LayerNorm one qc pair off the fc psum; normalize on ACT
                (Identity w/ scale+bias) or DVE (tensor_scalar)."""
                for i in range(2):
                    qc = 2 * p_ + i
                    sl = ps[:, i * F:(i + 1) * F]
                    nc.vector.bn_stats(st_t[:, qc, :], sl)
                    nc.vector.bn_aggr(mv_t[:, qc, :], st_t[:, qc, :])
                q0 = 2 * p_
                nc.scalar.activation(
                    rstd[:, q0:q0 + 2], mv_t[:, q0:q0 + 2, 1],
                    AF.Sqrt, bias=eps_t, scale=1.0,
                )
                nc.vector.reciprocal(rstd[:, q0:q0 + 2], rstd[:, q0:q0 + 2])
                if on_scalar:
                    nc.vector.tensor_mul(
                        nbias[:, q0:q0 + 2], mv_t[:, q0:q0 + 2, 0],
                        rstd[:, q0:q0 + 2],
                    )
                    nc.vector.tensor_scalar_mul(
                        nbias[:, q0:q0 + 2], nbias[:, q0:q0 + 2], negone
                    )
                for i in range(2):
                    qc = 2 * p_ + i
                    if on_scalar:
                        nc.scalar.activation(
                            o_sb[:, qc, :],
                            ps[:, i * F:(i + 1) * F],
                            AF.Identity,
                            bias=nbias[:, qc:qc + 1],
                            scale=rstd[:, qc:qc + 1],
                        )
                    else:
                        nc.vector.tensor_scalar(
                            o_sb[:, qc, :],
                            ps[:, i * F:(i + 1) * F],
                            mv_t[:, qc, 0:1],
                            rstd[:, qc:qc + 1],
                            op0=ALU.subtract,
                            op1=ALU.mult,
                        )
                eng = nc.sync if p_ == 0 else nc.scalar
                eng.dma_start(
                    out[:, q0:q0 + 2, :], o_sb[:, q0:q0 + 2, :]
                )

            # ---- G0: prologue + software-pipelined loop (scores for kc+1
            # are emitted before PV of kc so dripped projections never sit
            # between a scores matmul and its EXP on the PE queue) ----
            G0, G1 = (0, 1), (2, 3)
            ctxps0 = [
                cpsum.tile([P, QB], f32, tag="ctxp", name=f"ctxp{hh}")
                for hh in G0
            ]
            proj_q(0, on_scalar=True)
            proj_k(0, 0, split_cast=True)
            pt_cur = attn_sc(G0, 0)
            attn_band(G0, 0, pt_cur)
            proj_v(0)
            proj_v(1)
            proj_v(2)

            post = {
                0: [lambda: proj_v(3)],
                1: [lambda: proj_k(1, 0)],
                2: [lambda: proj_v(4), lambda: proj_v(5)],
                3: [lambda: proj_v(6)],
                4: [lambda: proj_v(7), lambda: proj_k(2, 0)],
                5: [lambda: proj_v(8), lambda: proj_v(9)],
                6: [lambda: proj_q(1), lambda: proj_v(10)],
                7: [lambda: proj_v(11), lambda: proj_k(3, 0)],
                8: [lambda: proj_v(12)],
                9: [lambda: proj_v(13), lambda: proj_k(0, 1)],
                10: [lambda: proj_v(14)],
                11: [lambda: proj_v(15), lambda: proj_k(1, 1)],
            }
            for kc in range(KC):
                if kc + 1 < KC:
                    pt_nxt = attn_sc(G0, kc + 1)
                    attn_band(G0, kc + 1, pt_nxt)
                attn_pv(G0, ctxps0, kc, pt_cur)
                pt_cur = pt_nxt
                for step in post.get(kc, []):
                    step()

            # ---- G0 -> G1 transition: 5-chunk score/exp window over the
            # drain so the psum handoff hides behind the EXP stream ----
            pts = []
            for kc in range(5):
                pt = attn_sc(G1, kc)
                attn_band(G1, kc, pt)
                pts.append(pt)
                if kc == 1:
                    e_zrows(G0, ctxps0, on_scalar=False)
                elif kc == 2:
                    e_ctxcopy(G0, ctxps0)
                elif kc == 3:
                    e_recip(G0)
            ctxps1 = [
                cpsum.tile([P, QB], f32, tag="ctxp", name=f"ctxp{hh}")
                for hh in G1
            ]
            for kc in range(4):
                attn_pv(G1, ctxps1, kc, pts[kc])
            pt_cur = pts[4]

            # ---- G1 pipelined loop with G0-epilogue drip ----
            drip = {
                4: [lambda: proj_k(2, 1)],
                5: [lambda: e_zscale(G0)],
                6: [lambda: proj_k(3, 1)],
                7: [lambda: nc.vector.tensor_copy(fcacc, res_sb)],
                9: [lambda: e_fc0_pair(0)],
                11: [lambda: e_fc0_pair(1)],
            }
            for kc in range(4, KC):
                if kc + 1 < KC:
                    pt_nxt = attn_sc(G1, kc + 1)
                    attn_band(G1, kc + 1, pt_nxt)
                attn_pv(G1, ctxps1, kc, pt_cur)
                pt_cur = pt_nxt
                for step in drip.get(kc, []):
                    step()

            # ---- G1 epilogue ----
            e_zrows(G1, ctxps1)
            c_ = RECIP_APPROX_FAST_CONSTS
            nc.vector._custom_dve(
                RECIPROCAL_APPROX_FAST, out=rz16[0:1, 2, :],
                in0=ztmp32[0:1, 2, :],
                s0=c_["s0"], s1=c_["s1"], imm2=c_["imm2"],
            )
            nc.vector.tensor_copy(ctx_sb[0:64, 1, :], ctxps1[0][0:64, :])
            nc.scalar.copy(ctx_sb[64:128, 1, :], ctxps1[1][0:64, :])
            nc.vector._custom_dve(
                RECIPROCAL_APPROX_FAST, out=rz16[0:1, 3, :],
                in0=ztmp32[0:1, 3, :],
                s0=c_["s0"], s1=c_["s1"], imm2=c_["imm2"],
            )
            e_zscale(G1, pool=cpsum)
            ps0 = e_fc1_mm(0)
            ps1 = e_fc1_mm(1)
            e_fc1_ln(0, ps0, on_scalar=True)
            e_fc1_ln(1, ps1, on_scalar=False)

    nc.compile()
    return nc


def get_nc():
    if "nc" not in _CACHE:
        _CACHE["nc"] = _build_program()
    return _CACHE["nc"]


def make_in_maps(input_Q, input_K, input_V, W_Q, W_K, W_V, W_fc):
    import ml_dtypes

    f8 = ml_dtypes.float8_e4m3
    c16 = lambda a: np.ascontiguousarray(
        np.asarray(a, dtype=np.float32), dtype=np.float16
    )
    # pack an [in, out] matrix to SBUF layout [p, c, out]
    pk_w = lambda w: np.asarray(w, np.float32).reshape(2, P, -1).transpose(1, 0, 2)
    # pack an activation block X [seq, F] to X^T SBUF layout [p, c, seq]
    pk_t = lambda x: c16(np.asarray(x, np.float32).T.reshape(2, P, -1).transpose(1, 0, 2))
    # pack a rolled key/value matrix [2048, F] to X^T [p, nb, c, 512]
    pk_x = lambda x: np.ascontiguousarray(
        np.asarray(x, np.float32).reshape(4, 512, 2, P).transpose(3, 0, 2, 1),
        dtype=f8,
    )
    e01t, e25t = _gauss_tables()
    g01 = np.ascontiguousarray(e01t.transpose(1, 0, 2))
    g25 = np.ascontiguousarray(e25t.transpose(1, 0, 2))
    g01_neutral = np.ones_like(g01)
    w8 = np.ascontiguousarray(
        np.stack([pk_w(W_K), pk_w(W_Q), pk_w(W_V)], axis=1), dtype=f8
    )
    wf = c16(pk_w(W_fc))
    in_maps = []
    for c in range(N_CORES):
        b, qb = divmod(c, 4)
        q0 = qb * QB
        r = (q0 - 256) % S
        xq_blk = np.asarray(input_Q[b][q0:q0 + QB], np.float32)
        xk_rot = np.roll(np.asarray(input_K[b], np.float32), -r, axis=0)
        xv_rot = np.roll(np.asarray(input_V[b], np.float32), -r, axis=0)
        in_maps.append({
            "xkt": pk_x(xk_rot),
            "xqt": np.ascontiguousarray(pk_t(xq_blk), dtype=f8),
            "g01": g01_neutral if q0 == 0 else g01,
            "g25": g25,
            "res": c16(xq_blk.reshape(4, P, F).transpose(1, 0, 2)),
            "w8": w8,
            "wf": wf,
            "xvt": pk_x(xv_rot),
        })
    return in_maps


def assemble_out(results):
    out = np.empty((B, S, F), dtype=np.float32)
    for c in range(N_CORES):
        b, qb = divmod(c, 4)
        o = np.asarray(results[c]["out"], dtype=np.float32)
        out[b, qb * QB:(qb + 1) * QB, :] = o.transpose(1, 0, 2).reshape(QB, F)
    return out


def kernel(input_Q, input_K, input_V, W_Q, W_K, W_V, W_fc, attn_mask=None):
    from concourse.bass_utils import run_bass_kernel_spmd

    nc = get_nc()
    in_maps = make_in_maps(input_Q, input_K, input_V, W_Q, W_K, W_V, W_fc)
    res = run_bass_kernel_spmd(nc, in_maps, core_ids=list(range(N_CORES)))
    return assemble_out(res.results)
